# revision 1
# baseline (speedup 1.0000x reference)
"""CKY kernel for 8x Trainium2 NeuronCores.

emissions [32,128,128,128] f32 -> logZ [32] f32 (CKY inside log-partition of
data = logsumexp(emissions, -1)).

Design (v6):
 - Data parallel: 4 sentences/core on 8 cores.
 - DP in scaled probability space (chart stores exp(t - ALPHA*p - DELTA)):
   the per-width logsumexp recurrence becomes pure multiply+add.
 - Phase 1: upper-triangle loads 8 diagonals per DMA, wide exp on ACT
   (f32 -> bf16), one 8-segment DVE reduce per tile. Diagonal-group tiles
   are partition-pair packed (group g on top, group 16-g below) so every
   ACT/reduce column is full-height: engine cost is proportional to free
   columns, so packing halves phase-1 engine time.
 - DP per width w: F1[w] = m1 + m2 + Rd with the span emission folded in:
     m1 = shift1(F1[w-1]) * C1[:, w]     (v=0;  C1 = K1*D0(s)*D(s,s+w))
     m2 = F1[w-1] * C2[:, w-1]           (u=0;  C2 = K1*D0(s+w)*D(s,s+w))
     Rd = (sum_v F1[v] * psM[v]) * dcol  (v in [1,w-2]: children >= 2 steps
                                          old -> off the critical path)
   The v-sum is a fused multiply+reduce (scalar_tensor_tensor accum_out)
   per batch lane, split across DVE and Pool. C2 uses a Hankel table
   D0S[s,k] = K1*D0[s+k+1] built with one DMA round trip through
   zero-padded DRAM scratch. F2 (dual chart, bf16) is maintained by a PE
   shift into PSUM + Pool copy: no DMA, and rows < w come out of the
   matmul as zeros which is the correct chart value there. The wide psM
   shift matmuls run in bf16 (~4x cheaper than f32 on the PE).
"""
import os
import sys

sys.path.insert(0, "/opt/trn_rl_repo")

import numpy as np

import concourse.bacc as bacc
import concourse.mybir as mybir
import concourse.tile as tile
from concourse import bass_utils
from concourse.ap import AP

ALPHA = 12.05
DELTA = -10.9
N = 128
M = 128
BC = 4
NCORES = 8
G = 8
NG = N // G
_f32 = mybir.dt.float32
_bf16 = mybir.dt.bfloat16


def _build_cky(tc, out_dram, em_dram, red_mode="dve", uid=""):
    """v5 baseline (kept for A/B): PE-shift DP, F2 column writes via DMA."""
    nc = tc.nc
    K1 = float(np.exp(-ALPHA - 2 * DELTA))

    with tc.tile_pool(name="pers", bufs=1) as pers, \
         tc.tile_pool(name="st", bufs=6) as stp, \
         tc.tile_pool(name="psum", bufs=2, space="PSUM") as psp:

        F1 = pers.tile([N, BC * N], _f32)
        F2 = pers.tile([N, BC * N], _f32)
        nc.gpsimd.memset(F1[:], 0.0)
        nc.gpsimd.memset(F2[:], 0.0)
        Dg = [pers.tile([N, BC * G], _f32, tag=f"dred{g}",
                        name=f"{uid}dred{g}") for g in range(NG)]
        biasap = pers.tile([N, 1], _f32)
        nc.vector.memset(biasap[:], DELTA)

        for g in range(NG):
            for b in range(BC):
                base = b * N * N * M
                st = stp.tile([N, G * M], _f32, tag="st")
                L = N - G * g
                ragged = (g == 0 and b == BC - 1)
                if ragged:
                    L = N - (G - 1)
                    nc.gpsimd.memset(st[:], 0.0)
                src = AP(em_dram.tensor, base + G * g * M,
                         [[(N + 1) * M, L], [1, G * M]])
                nc.sync.dma_start(st[0:L, :], src)
                if ragged:
                    for s in range(N - G + 1, N):
                        cnt = (N - s) * M
                        fsrc = AP(em_dram.tensor, base + (s * N + s) * M,
                                  [[cnt, 1], [1, cnt]])
                        nc.sync.dma_start(st[s:s + 1, 0:cnt], fsrc)
                LE = N if ragged else L
                nc.scalar.activation(st[0:LE, :], st[0:LE, :],
                                     mybir.ActivationFunctionType.Exp,
                                     bias=biasap[0:LE, :], scale=1.0)
                st3 = st.rearrange("s (c m) -> s c m", c=G)
                dg3 = Dg[g].rearrange("s (c b) -> s c b", c=G)
                nc.vector.reduce_sum(dg3[0:LE, :, b], st3[0:LE],
                                     axis=mybir.AxisListType.X)

        nc.vector.tensor_scalar_mul(F1[:, 0:BC], Dg[0][:, 0:BC], K1)
        nc.vector.tensor_scalar_mul(F2[:, (N - 1) * BC:N * BC],
                                    Dg[0][:, 0:BC], K1)

        Wm = pers.tile([N, 3 * N], _f32)
        nc.gpsimd.memset(Wm[:], 0.0)
        nc.gpsimd.affine_select(out=Wm[:], in_=Wm[:],
                                compare_op=mybir.AluOpType.not_equal,
                                fill=1.0, base=N,
                                pattern=[[-1, 3 * N]], channel_multiplier=1)

        prod = pers.tile([N, BC * N], _f32)
        red = pers.tile([N, BC], _f32)
        red2 = pers.tile([N, BC], _f32)
        t0 = pers.tile([N, BC], _f32)
        Dgv = [d.rearrange("s (c b) -> s c b", c=G) for d in Dg]

        for w in range(1, N):
            L = N - w
            dcol = Dgv[w // G][0:L, (w % G), :]
            ps1 = psp.tile([N, BC], _f32, tag="ps1", name=f"{uid}ps1_{w}")
            nc.tensor.matmul(ps1[:], Wm[:, N + 1:2 * N + 1],
                             F1[:, (w - 1) * BC:w * BC])
            if w >= 2:
                psS = psp.tile([N, BC * N], _f32, tag="psS",
                               name=f"{uid}psS_{w}")
                nc.tensor.matmul(psS[:, BC:w * BC], Wm[:, N + w:2 * N + w],
                                 F2[:, (N - w + 1) * BC:N * BC])
                nc.vector.tensor_mul(prod[0:L, BC:w * BC],
                                     F1[0:L, BC:w * BC],
                                     psS[0:L, BC:w * BC])
                pr3 = prod.rearrange("s (v b) -> s b v", b=BC)
                nc.vector.reduce_sum(red[0:L, :], pr3[0:L, :, 1:w],
                                     axis=mybir.AxisListType.X)
            nc.vector.tensor_mul(t0[0:L, :], F1[0:L, 0:BC], ps1[0:L, :])
            if w >= 2:
                nc.vector.tensor_add(red2[0:L, :], red[0:L, :], t0[0:L, :])
            else:
                nc.vector.tensor_copy(red2[0:L, :], t0[0:L, :])
            nc.vector.tensor_mul(F1[0:L, w * BC:(w + 1) * BC],
                                 red2[0:L, :], dcol)
            if w < N - 1:
                nc.sync.dma_start(F2[w:N, (N - 1 - w) * BC:(N - w) * BC],
                                  F1[0:L, w * BC:(w + 1) * BC])

        nc.sync.dma_start(out_dram[:], F1[0:1, (N - 1) * BC:N * BC])


def _build_cky_v6(tc, out_dram, em_dram, uid=""):
    nc = tc.nc
    K1 = float(np.exp(-ALPHA - 2 * DELTA))
    NB = N * BC
    # runtime-crash bisect knobs
    no_bf16 = os.environ.get("CKY_NO_BF16", "0") == "1"
    no_actcopy = os.environ.get("CKY_NO_ACTCOPY", "0") == "1"
    f2_dt = _f32 if no_bf16 else _bf16

    d0d = nc.dram_tensor(f"d0scratch{uid}", [2 * NB], _f32, kind="Internal")

    with tc.tile_pool(name="pers", bufs=1) as pers, \
         tc.tile_pool(name="st", bufs=4) as stp, \
         tc.tile_pool(name="st2", bufs=4) as st2p, \
         tc.tile_pool(name="pms", bufs=2) as pmsp, \
         tc.tile_pool(name="psum", bufs=2, space="PSUM") as psp:

        F1 = pers.tile([N, NB], _f32)
        F2 = pers.tile([N, NB], f2_dt)
        Dall = pers.tile([N, NB], _f32)
        C1 = pers.tile([N, NB], _f32)
        C2 = pers.tile([N, NB], _f32)
        D0S = pers.tile([N, NB], _f32)
        prodA = pers.tile([N, NB], _f32)
        prodB = pers.tile([N, NB], _f32)
        nc.gpsimd.memset(F1[:], 0.0)
        nc.gpsimd.memset(F2[:], 0.0)
        biasap = pers.tile([N, 1], _f32)
        nc.vector.memset(biasap[:], DELTA)

        Dall3 = Dall.rearrange("s (w b) -> s w b", b=BC)
        C13 = C1.rearrange("s (w b) -> s w b", b=BC)
        C23 = C2.rearrange("s (w b) -> s w b", b=BC)

        # shift matrix: Wm[k, c] = 1 iff c == k + N; slice [N+d : 2N+d] as
        # matmul lhsT shifts partitions: out[m, :] = X[m+d, :] (0 if OOB).
        Wm = pers.tile([N, 3 * N], _f32)
        nc.gpsimd.memset(Wm[:], 0.0)
        nc.gpsimd.affine_select(out=Wm[:], in_=Wm[:],
                                compare_op=mybir.AluOpType.not_equal,
                                fill=1.0, base=N,
                                pattern=[[-1, 3 * N]], channel_multiplier=1)
        # bf16 copies: bf16 matmul is ~4x cheaper and 0/1 entries are
        # exact. A bf16 lhsT slice must start 4-byte aligned, i.e. at an
        # even element offset: slice [N+w : 2N+w] is odd for odd w, which
        # crashes the device (NRT_EXEC_UNIT_UNRECOVERABLE). WmbO carries
        # the 1-diagonal at c == k + N + 1, so odd-w shifts use the
        # even-offset slice [N+1+w : 2N+1+w] of WmbO instead.
        Wmb = pers.tile([N, 3 * N], f2_dt)
        nc.gpsimd.tensor_copy(Wmb[:], Wm[:])
        WmO = pers.tile([N, 3 * N], _f32)
        nc.gpsimd.memset(WmO[:], 0.0)
        nc.gpsimd.affine_select(out=WmO[:], in_=WmO[:],
                                compare_op=mybir.AluOpType.not_equal,
                                fill=1.0, base=N + 1,
                                pattern=[[-1, 3 * N]], channel_multiplier=1)
        WmbO = pers.tile([N, 3 * N], f2_dt)
        nc.gpsimd.tensor_copy(WmbO[:], WmO[:])

        K1D0 = pers.tile([N, BC], _f32)
        K1D0r = pers.tile([N, G * BC], _f32)
        zt = pers.tile([N, BC], _f32)
        m1t = pers.tile([N, BC], _f32)
        m2t = pers.tile([N, BC], _f32)
        Pt = pers.tile([N, BC], _f32)
        RmA = pers.tile([N, BC], _f32)
        RmB = pers.tile([N, BC], _f32)
        RdA = pers.tile([N, BC], _f32)
        RdB = pers.tile([N, BC], _f32)

        def _load_rows(st, g, b, row_off, rows):
            base = b * N * N * M
            src = AP(em_dram.tensor, base + G * g * M,
                     [[(N + 1) * M, rows], [1, G * M]])
            nc.sync.dma_start(st[row_off:row_off + rows, :], src)

        def _load_group(st, g, b, row_off):
            # rows [row_off : row_off + (N-8g)] <- group g of sentence b;
            # g0/b3 is the one truly OOB tile and gets ragged fixups.
            L = N - G * g
            ragged = (g == 0 and b == BC - 1)
            if ragged:
                L = N - (G - 1)
                nc.gpsimd.memset(st[:], 0.0)
            _load_rows(st, g, b, row_off, L)
            if ragged:
                base = b * N * N * M
                for s in range(L, N):
                    cnt = (N - s) * M
                    fsrc = AP(em_dram.tensor, base + (s * N + s) * M,
                              [[cnt, 1], [1, cnt]])
                    nc.sync.dma_start(st[s:s + 1, 0:cnt], fsrc)

        def _exp_reduce(st, out_ap):
            # wide exp (f32 -> bf16) + one 8-segment reduce over all rows.
            # Junk in padded / overrun regions only ever lands in chart
            # cells (s, w) with s + w >= N, which the DP never reads.
            st2 = st2p.tile([N, G * M], _bf16, tag="st2")
            nc.scalar.activation(st2[:, :], st[:, :],
                                 mybir.ActivationFunctionType.Exp,
                                 bias=biasap[:, :], scale=1.0)
            st3 = st2.rearrange("s (c m) -> s c m", c=G)
            nc.vector.reduce_sum(out_ap, st3[:], axis=mybir.AxisListType.X)

        def _phase1_g0():
            for b in range(BC):
                st = stp.tile([N, G * M], _f32, tag="st")
                _load_group(st, 0, b, 0)
                _exp_reduce(st, Dall3[:, 0:G, b])

        def _phase1_pair(g):
            # partition-pair packing: group g rows [0:128-8g] on top, group
            # 16-g (8g rows) below -> full-height ACT/reduce columns.
            g2 = 16 - g
            Lg = N - G * g
            for b in range(BC):
                st = stp.tile([N, G * M], _f32, tag="st")
                _load_group(st, g, b, 0)
                _load_group(st, g2, b, Lg)
                Sg = st2p.tile([N, G], _f32, tag="sg")
                _exp_reduce(st, Sg[:, :])
                # unpack: top rows belong to group g, bottom to group g2
                nc.gpsimd.tensor_copy(Dall3[0:Lg, G * g:G * g + G, b],
                                      Sg[0:Lg, :])
                psU = psp.tile([N, G], _f32, tag="psU",
                               name=f"{uid}psU_{g}_{b}")
                nc.tensor.matmul(psU[:], Wm[:, N + Lg:2 * N + Lg], Sg[:, :])
                # Pool (GPSIMD) cannot access PSUM on HW: psU reads go to DVE
                nc.vector.tensor_copy(Dall3[0:8 * g, G * g2:G * g2 + G, b],
                                      psU[0:8 * g, :])

        def _phase1_g8():
            # group 8 is 64 rows: pack two sentences per tile
            for b in (0, 2):
                st = stp.tile([N, G * M], _f32, tag="st")
                _load_group(st, 8, b, 0)
                _load_group(st, 8, b + 1, 64)
                S8 = st2p.tile([N, G], _f32, tag="sg")
                _exp_reduce(st, S8[:, :])
                nc.gpsimd.tensor_copy(Dall3[0:64, 64:72, b], S8[0:64, :])
                psU = psp.tile([N, G], _f32, tag="psU",
                               name=f"{uid}psU8_{b}")
                nc.tensor.matmul(psU[:], Wm[:, N + 64:2 * N + 64], S8[:, :])
                nc.vector.tensor_copy(Dall3[0:64, 64:72, b + 1],
                                      psU[0:64, :])

        def _d0_prep():
            # K1*D0 -> DRAM (zero-padded) -> Hankel D0S[s,k] = K1*D0[s+k+1]
            nc.vector.tensor_scalar_mul(K1D0[:], Dall[:, 0:BC], K1)
            nc.gpsimd.memset(zt[:], 0.0)
            nc.sync.dma_start(AP(d0d, 0, [[BC, N], [1, BC]]), K1D0[:])
            nc.sync.dma_start(AP(d0d, NB, [[BC, N], [1, BC]]), zt[:])
            nc.sync.dma_start(D0S[:], AP(d0d, BC, [[BC, N], [1, NB]]))
            # K1D0r: K1D0 replicated 8x along the free axis (for C1 blocks)
            nc.gpsimd.tensor_copy(K1D0r[:, 0:BC], K1D0[:])
            nc.gpsimd.tensor_copy(K1D0r[:, BC:2 * BC], K1D0[:])
            nc.gpsimd.tensor_copy(K1D0r[:, 2 * BC:4 * BC], K1D0r[:, 0:2 * BC])
            nc.gpsimd.tensor_copy(K1D0r[:, 4 * BC:8 * BC], K1D0r[:, 0:4 * BC])
            # width-0 init
            nc.vector.tensor_copy(F1[:, 0:BC], K1D0[:])
            nc.gpsimd.tensor_copy(F2[:, (N - 1) * BC:N * BC], K1D0[:])

        def _c1c2_group(g):
            # C1[:, w] = K1*D0[s] * D(s,s+w), w in group g
            lo, hi = 8 * g * BC, (8 * g + 8) * BC
            nc.gpsimd.tensor_mul(C1[:, lo:hi], Dall[:, lo:hi], K1D0r[:])
            # C2[:, k] = D0S[:, k] * D(s, s+k+1), k in [8g-1, 8g+7)
            klo = (8 * g - 1) * BC if g > 0 else 0
            khi = (8 * g + 7) * BC
            nc.gpsimd.tensor_mul(C2[:, klo:khi], D0S[:, klo:khi],
                                 Dall[:, klo + BC:khi + BC])

        def _dp_early(w):
            L = N - w
            even = (w % 2 == 0)
            prod = prodA if even else prodB
            Rm = RmA if even else RmB
            Rd = RdA if even else RdB

            if w >= 3:
                # mid terms v in [1, w-2]: psM[m, v] = F2[m+w, N-w+v].
                psM = psp.tile([N, NB], _f32, tag="psM",
                               name=f"{uid}psM_{w}")
                lhs = (Wmb[:, N + w:2 * N + w] if w % 2 == 0 else
                       WmbO[:, N + 1 + w:2 * N + 1 + w])
                nc.tensor.matmul(psM[:, BC:(w - 1) * BC], lhs,
                                 F2[:, (N - w + 1) * BC:(N - 1) * BC])
                # Pool (GPSIMD) cannot access PSUM on HW and codegen
                # rejects scalar_tensor_tensor on Pool: stage the shifted
                # chart into SBUF via ACT (mostly idle during DP), multiply
                # on Pool, reduce on DVE.
                psMs = pmsp.tile([N, NB], _f32, tag="psms")
                if no_actcopy:
                    nc.vector.tensor_copy(psMs[:, BC:(w - 1) * BC],
                                          psM[:, BC:(w - 1) * BC])
                else:
                    nc.scalar.activation(psMs[:, BC:(w - 1) * BC],
                                         psM[:, BC:(w - 1) * BC],
                                         mybir.ActivationFunctionType.Copy,
                                         bias=0.0, scale=1.0)
                nc.gpsimd.tensor_mul(prod[0:L, BC:(w - 1) * BC],
                                     F1[0:L, BC:(w - 1) * BC],
                                     psMs[0:L, BC:(w - 1) * BC])
                pr3 = prod.rearrange("s (v b) -> s b v", b=BC)
                nc.vector.reduce_sum(Rm[0:L, :], pr3[0:L, :, 1:w - 1],
                                     axis=mybir.AxisListType.X)
                nc.gpsimd.tensor_mul(Rd[0:L, :], Rm[0:L, :], Dall3[0:L, w, :])

        def _dp_late(w):
            L = N - w
            even = (w % 2 == 0)
            Rd = RdA if even else RdB

            # F2 maintenance for width w-1 (needed by psM at step w+1)
            if 1 <= w - 1 <= N - 3:
                v2 = w - 1
                psB = psp.tile([N, BC], _f32, tag="psB",
                               name=f"{uid}psB_{w}")
                nc.tensor.matmul(psB[:], Wm[:, N - v2:2 * N - v2],
                                 F1[:, v2 * BC:(v2 + 1) * BC])
                nc.vector.tensor_copy(F2[:, (N - 1 - v2) * BC:(N - v2) * BC],
                                      psB[:])

            # critical path (m2/P on Pool: all-SBUF operands, rebalances
            # DVE which carries the phase-1 reduces + mid reduces)
            if w >= 2:
                nc.gpsimd.tensor_mul(m2t[0:L, :],
                                     F1[0:L, (w - 1) * BC:w * BC],
                                     C23[0:L, w - 1, :])
                if w >= 3:
                    nc.gpsimd.tensor_add(Pt[0:L, :], m2t[0:L, :], Rd[0:L, :])
            psA = psp.tile([N, BC], _f32, tag="psA", name=f"{uid}psA_{w}")
            nc.tensor.matmul(psA[:], Wm[:, N + 1:2 * N + 1],
                             F1[:, (w - 1) * BC:w * BC])
            if w == 1:
                nc.vector.tensor_mul(F1[0:L, w * BC:(w + 1) * BC],
                                     psA[0:L, :], C13[0:L, w, :])
            else:
                nc.vector.tensor_mul(m1t[0:L, :], psA[0:L, :], C13[0:L, w, :])
                src2 = Pt if w >= 3 else m2t
                nc.vector.tensor_add(F1[0:L, w * BC:(w + 1) * BC],
                                     m1t[0:L, :], src2[0:L, :])

        # ---- interleaved emission: unit u makes group u (and 16-u)
        # available; DP width block [8(u-1), 8u) needs groups <= u-1.
        # Early work (psM/psMs/mul/reduce/Rd, all deps >= 2 steps old) is
        # emitted LOOK steps ahead of the late chain so the PE/ACT/Pool/DVE
        # queues never head-of-line block behind chain-dependent ops. ----
        def _dp_step(w):
            _dp_early(w)
            _dp_late(w)

        for u in range(9):
            if u == 0:
                _phase1_g0()
                _d0_prep()
                _c1c2_group(0)
            elif u == 8:
                _phase1_g8()
                _c1c2_group(8)
            else:
                _phase1_pair(u)
                _c1c2_group(u)
                _c1c2_group(16 - u)
            if u >= 1:
                for w in range(max(1, 8 * (u - 1)), 8 * u):
                    _dp_step(w)
        for w in range(64, N):
            _dp_step(w)

        nc.sync.dma_start(out_dram[:], F1[0:1, (N - 1) * BC:N * BC])


_CACHE: dict = {}


def _get_nc(reps=1):
    key = f"nc{reps}"
    if key not in _CACHE:
        nc = bacc.Bacc("TRN2", target_bir_lowering=False, debug=False,
                       enable_asserts=False, num_devices=NCORES)
        em = nc.dram_tensor("emissions", [BC, N, N, M], _f32,
                            kind="ExternalInput")
        out = nc.dram_tensor("out", [BC], _f32, kind="ExternalOutput")
        with tile.TileContext(nc) as tc:
            for r in range(reps):
                uid = f"r{r}_" if reps > 1 else ""
                if os.environ.get("CKY_KERNEL", "v6") == "v6":
                    _build_cky_v6(tc, out.ap(), em.ap(), uid=uid)
                else:
                    _build_cky(tc, out.ap(), em.ap(), uid=uid)
        nc.compile()
        _CACHE[key] = nc
    return _CACHE[key]


def _run(emissions, **spmd_kwargs):
    emissions = np.ascontiguousarray(emissions, dtype=np.float32)
    assert emissions.shape == (BC * NCORES, N, N, M)
    nc = _get_nc()
    in_maps = [{"emissions": emissions[c * BC:(c + 1) * BC]}
               for c in range(NCORES)]
    res = bass_utils.run_bass_kernel_spmd(nc, in_maps,
                                          core_ids=list(range(NCORES)),
                                          **spmd_kwargs)
    outs = np.concatenate([res.results[c]["out"] for c in range(NCORES)])
    logz = np.log(outs.astype(np.float64)) + (ALPHA * N + DELTA)
    return logz.astype(np.float32), res


def kernel(emissions):
    logz, _ = _run(emissions)
    return logz


def kernel_bench_hw(emissions, R=9, reps=100):
    """On-device HW time via the slope method.

    Compiles two Bass modules: the kernel emitted once, and the kernel body
    emitted R times back-to-back in one NEFF (iterations serialize on-device
    through WAR hazards on the persistent chart tiles). Both programs pay
    the same axon-tunnel round trip and NEFF-invocation overhead; the
    wall-clock difference divided by (R-1) is pure per-execution device
    time.

    Returns (logZ, hw_ns_per_exec, t1_best_s, tR_best_s).
    """
    import time
    import jax
    from jax.sharding import Mesh, PartitionSpec, NamedSharding
    from jax.experimental.shard_map import shard_map
    from concourse import bass2jax

    emissions = np.ascontiguousarray(emissions, dtype=np.float32)
    bass2jax.install_neuronx_cc_hook()

    def _make_fn(nc):
        in_names, out_names, out_avals = [], [], []
        for alloc in nc.m.functions[0].allocations:
            if not isinstance(alloc, mybir.MemoryLocationSet):
                continue
            name = alloc.memorylocations[0].name
            if alloc.kind == "ExternalInput":
                if nc.partition_id_tensor is None or \
                        name != nc.partition_id_tensor.name:
                    in_names.append(name)
            elif alloc.kind == "ExternalOutput":
                out_names.append(name)
                shape = tuple(alloc.tensor_shape)
                dtype = mybir.dt.np(alloc.dtype)
                out_avals.append(jax.core.ShapedArray(shape, dtype))
        all_in_names = list(in_names) + list(out_names)
        if nc.partition_id_tensor is not None:
            all_in_names = all_in_names + [nc.partition_id_tensor.name]

        def _body(em, z):
            operands = [em, z]
            if nc.partition_id_tensor is not None:
                operands.append(bass2jax.partition_id_tensor())
            outs = bass2jax._bass_exec_p.bind(
                *operands, out_avals=tuple(out_avals),
                in_names=tuple(all_in_names), out_names=tuple(out_names),
                lowering_input_output_aliases=(),
                sim_require_finite=True, sim_require_nnan=True, nc=nc)
            return (outs[0],)

        mesh = Mesh(np.asarray(jax.devices()[:NCORES]), ("core",))
        return jax.jit(shard_map(
            _body, mesh=mesh,
            in_specs=(PartitionSpec("core"), PartitionSpec("core")),
            out_specs=(PartitionSpec("core"),), check_rep=False),
            keep_unused=True), mesh

    nc1 = _get_nc(1)
    ncR = _get_nc(R)
    fn1, mesh = _make_fn(nc1)
    fnR, _ = _make_fn(ncR)
    sharding = NamedSharding(mesh, PartitionSpec("core"))
    x = jax.device_put(emissions, sharding)
    z = jax.device_put(np.zeros((NCORES * BC,), np.float32), sharding)

    o1 = fn1(x, z)[0]; o1.block_until_ready()   # warm + compile
    oR = fnR(x, z)[0]; oR.block_until_ready()
    assert np.allclose(np.asarray(o1), np.asarray(oR), rtol=1e-6), \
        "repeated-body module diverged from single-body module"

    # Slope of best-of floors: with enough reps both programs hit the
    # tunnel-RTT floor, and floor_R - floor_1 is exactly (R-1) executions
    # of pure device time. (Paired differencing is NOT robust here: the
    # tunnel has slow drift plus per-call jitter larger than the signal.)
    t1 = float("inf"); tR = float("inf")
    for _ in range(reps):
        t0 = time.perf_counter(); fn1(x, z)[0].block_until_ready()
        t1 = min(t1, time.perf_counter() - t0)
        t0 = time.perf_counter(); fnR(x, z)[0].block_until_ready()
        tR = min(tR, time.perf_counter() - t0)
    hw_ns = (tR - t1) / (R - 1) * 1e9
    outs = np.asarray(o1)
    logz = np.log(outs.astype(np.float64)) + (ALPHA * N + DELTA)
    return logz.astype(np.float32), hw_ns, t1, tR

